# revision 22
# baseline (speedup 1.0000x reference)
"""Trainium2 Bass kernel for nn_DecoderBlock (B=4, S=2048, E=1024, H=16, D=64).

Sharding: 8 cores = 4 batches x 2 sequence-halves. Each core owns 1024 query
positions of one batch (a balanced causal split: core-even takes q [0:512)+
[1536:2048), core-odd takes q [512:1536)) and recomputes full-S K/V for its
batch locally (no collectives). Proj + FFN are token-parallel on the owned
1024 positions. Everything on-chip is in transposed layout (feature dim on
partitions); the host pre-transposes x and re-transposes the output.

The per-core program is identical (SPMD); per-core differences (which q
columns, causal masks) are encoded in the host-prepared inputs: xT columns
are reordered to [own-q | other-q], and causal masks are shipped per-core.
"""

import numpy as np
import ml_dtypes

import concourse.bass as bass
import concourse.tile as tile
from concourse import bacc, mybir
from concourse.bass_utils import run_bass_kernel_spmd

B, S, E, H, D = 4, 2048, 1024, 16, 64
QC = 1024          # queries owned per core
CH = 512           # q-chunk (matmul moving dim)
ET = E // 128      # 8 e-tiles
HT = (4 * E) // 128  # 32 ffn hidden tiles
SCALE = float(E) ** -0.5

F32R = mybir.dt.float32r
F32 = mybir.dt.float32
BF16 = mybir.dt.bfloat16

# Attention slot tables: (t_tile, mask_idx or None); uniform across cores.
# xT t-order is [own qA | own qB | other qA | other qB] (512 cols each).
# Each slot computes BOTH heads of a pair concurrently via PE row-groups
# (rows 0-63 / 64-127) into one [128,1024] psum tile -> one exp op.
CHUNK_A = [(0, 0), (1, 1), (2, 2), (3, 3), (8, 4), (9, 5), (10, 6), (11, 7)]
CHUNK_B = [(0, None), (1, None), (2, None), (3, None),
           (4, 8), (5, 9), (6, 10), (7, 11),
           (8, None), (9, None), (10, None), (11, None),
           (12, 12), (13, 13), (14, 14), (15, 15)]
N_MASKS = 16

_CACHE = {}
LAST_RESULTS = None


# V-unit emission order per half: chunk-A slot order first, then chunk-B extras
V_ORD = [0, 1, 2, 3, 8, 9, 10, 11, 4, 5, 6, 7, 12, 13, 14, 15]


def _phase1_attention(nc, tc, xt, at, mk, dram, preload=None):
    """Attention with interleaved K/Q/V filler units inside the slot stream.

    Per slot: two per-head scores matmuls [128t,512q] -> exp -> mask; AV runs
    with exp-scores as the STATIONARY operand (out [128q, 65], moving dim 65 =
    64 v-features + ones column for the softmax denominator), accumulated over
    slots per q-subtile. Chunk drain: reciprocal + per-subtile scale, then PE
    transpose back to [feat, q] into at. K/Q/V units for later pairs are
    emitted between slots so the PE never starves while ACT runs exp.
    """
    from functools import partial as F
    from concourse.masks import make_identity

    with (
        tc.tile_pool(name="wkq", bufs=3) as wkq_pool,
        tc.tile_pool(name="wv", bufs=2) as wv_pool,
        tc.tile_pool(name="kt", bufs=3) as kt_pool,
        tc.tile_pool(name="qt", bufs=3) as qt_pool,
        tc.tile_pool(name="vt", bufs=2) as vt_pool,
        tc.tile_pool(name="es", bufs=3) as es_pool,
        tc.tile_pool(name="an", bufs=2) as an_pool,
        tc.tile_pool(name="norm", bufs=2) as nm_pool,
        tc.tile_pool(name="ident", bufs=1) as id_pool,
        tc.tile_pool(name="ps_kqv", bufs=2, space="PSUM") as pp_kqv,
        tc.tile_pool(name="ps_s", bufs=2, space="PSUM") as pp_s,
        tc.tile_pool(name="ps_av", bufs=2, space="PSUM") as pp_av,
    ):
        ident = id_pool.tile([128, 128], BF16)
        make_identity(nc, ident[:])

        kts, qts, wks, wqs, vts, wvs = {}, {}, {}, {}, {}, {}

        def k_unit(p, tcnk):
            if tcnk == 0:
                if preload and ("wk", p) in preload:
                    wks[p] = preload[("wk", p)]
                else:
                    wks[p] = wkq_pool.tile([128, ET, 128], BF16, tag="w", name=f"wk{p}")
                    nc.sync.dma_start(wks[p][:], dram["wk"][p])
                kts[p] = kt_pool.tile([128, S], BF16, tag="kt", name=f"kt{p}")
            ps = pp_kqv.tile([128, CH], F32)
            for et in range(ET):
                nc.tensor.matmul(
                    ps[:], wks[p][:, et, :], xt[:, et, tcnk * CH:(tcnk + 1) * CH],
                    start=(et == 0), stop=(et == ET - 1))
            nc.vector.tensor_copy(kts[p][:, tcnk * CH:(tcnk + 1) * CH], ps[:])

        def q_unit(p, c):
            if c == 0:
                if preload and ("wq", p) in preload:
                    wqs[p] = preload[("wq", p)]
                else:
                    wqs[p] = wkq_pool.tile([128, ET, 128], BF16, tag="w", name=f"wq{p}")
                    nc.sync.dma_start(wqs[p][:], dram["wq"][p])
                qts[p] = qt_pool.tile([128, QC], BF16, tag="qt", name=f"qt{p}")
            ps = pp_kqv.tile([128, CH], F32)
            for et in range(ET):
                nc.tensor.matmul(
                    ps[:], wqs[p][:, et, :], xt[:, et, c * CH:(c + 1) * CH],
                    start=(et == 0), stop=(et == ET - 1))
            nc.vector.tensor_copy(qts[p][:, c * CH:(c + 1) * CH], ps[:])

        def v_unit(half, i):
            tt = V_ORD[i]
            if i == 0:
                wvs[half] = wv_pool.tile([128, ET, 512], BF16, tag="wv", name=f"wv{half}")
                nc.sync.dma_start(wvs[half][:], dram["wv"][half])
                vts[half] = vt_pool.tile([128, 16, 8, 65], BF16, tag="vt", name=f"vt{half}")
                nc.vector.memset(vts[half][:, :, :, 64:65], 1.0)
            ps = pp_kqv.tile([128, CH], F32)
            for et in range(ET):
                nc.tensor.matmul(
                    ps[:], xt[:, et, tt * 128:(tt + 1) * 128], wvs[half][:, et, :],
                    start=(et == 0), stop=(et == ET - 1))
            nc.vector.tensor_copy(
                vts[half][:, tt, :, 0:64],
                ps[:].rearrange("p (h d) -> p h d", h=8))

        def slot_front(p, c, tt, mi):
            """Scores + exp + masks for one slot; returns the es tile."""
            kt, qt = kts[p], qts[p]
            ps = pp_s.tile([128, 2 * CH], F32, tag="ps")
            nc.tensor.matmul(
                ps[:, 0:CH], kt[0:64, tt * 128:(tt + 1) * 128],
                qt[0:64, c * CH:(c + 1) * CH], start=True, stop=True)
            nc.tensor.matmul(
                ps[:, CH:2 * CH], kt[64:128, tt * 128:(tt + 1) * 128],
                qt[64:128, c * CH:(c + 1) * CH], start=True, stop=True)
            es = es_pool.tile([128, 2 * CH], BF16)
            nc.scalar.activation(
                es[:], ps[:], mybir.ActivationFunctionType.Exp, scale=SCALE)
            if mi is not None:
                nc.vector.tensor_mul(es[:, 0:CH], es[:, 0:CH], mk[:, mi, :])
                nc.vector.tensor_mul(
                    es[:, CH:2 * CH], es[:, CH:2 * CH], mk[:, mi, :])
            return es

        def slot_avs(p, si, n, es, tt, av0, av1):
            # One accumulation group per av bank: start zeroes the whole 2KB
            # zero region, so only (si=0, sub=0) starts and (last si, sub=3)
            # stops; the 4 subtile chains share the group.
            vt = vts[p // 4]
            hh0, hh1 = 2 * (p % 4), 2 * (p % 4) + 1
            for sub in range(4):
                st = si == 0 and sub == 0
                sp = si == n - 1 and sub == 3
                nc.tensor.matmul(
                    av0[:, sub, 0:65], es[:, sub * 128:(sub + 1) * 128],
                    vt[:, tt, hh0, :], start=st, stop=sp, skip_group_check=True)
                nc.tensor.matmul(
                    av1[:, sub, 0:65], es[:, CH + sub * 128:CH + (sub + 1) * 128],
                    vt[:, tt, hh1, :], start=st, stop=sp, skip_group_check=True)

        def drain_dve(av0, av1):
            """Normalize: reciprocal of denominators + scale to bf16 an."""
            rc = nm_pool.tile([128, 2, 4], F32, tag="rc")
            nc.vector.reciprocal(rc[:, 0, :], av0[:, :, 64])
            nc.vector.reciprocal(rc[:, 1, :], av1[:, :, 64])
            an = an_pool.tile([128, 4, 2, 64], BF16)
            for sub in range(4):
                nc.vector.tensor_scalar_mul(
                    an[:, sub, 0, :], av0[:, sub, 0:64], rc[:, 0, sub:sub + 1])
                nc.vector.tensor_scalar_mul(
                    an[:, sub, 1, :], av1[:, sub, 0:64], rc[:, 1, sub:sub + 1])
            return an

        def drain_pe(p, c, an):
            """Transpose an back to [feat, q] and store into at."""
            for sub in range(4):
                tr = pp_kqv.tile([128, 128], BF16, tag="ps", name="tr")
                nc.tensor.transpose(
                    tr[:], an[:, sub].rearrange("p h d -> p (h d)"), ident[:])
                nc.vector.tensor_copy(
                    at[:, p, c * CH + sub * 128:c * CH + (sub + 1) * 128], tr[:])

        def pair_units(p):
            return [F(k_unit, p, 0), F(q_unit, p, 0), F(k_unit, p, 1),
                    F(k_unit, p, 2), F(k_unit, p, 3), F(q_unit, p, 1)]

        def emit_pair(p, fill):
            fi, nf = 0, len(fill)
            pend_avs, pend_tr = None, None
            for c, slots in ((0, CHUNK_A), (1, CHUNK_B)):
                av0 = pp_av.tile([128, 4, 128], F32, tag="av")
                av1 = pp_av.tile([128, 4, 128], F32, tag="av")
                n = len(slots)
                gbase = 0 if c == 0 else len(CHUNK_A)
                for si, (tt, mi) in enumerate(slots):
                    es = slot_front(p, c, tt, mi)
                    if pend_avs is not None:
                        pend_avs()
                        pend_avs = None
                    if pend_tr is not None:
                        pend_tr()
                        pend_tr = None
                    pend_avs = F(slot_avs, p, si, n, es, tt, av0, av1)
                    due = min(nf, ((gbase + si + 1) * nf) // 24)
                    while fi < due:
                        fill[fi]()
                        fi += 1
                pend_avs()
                pend_avs = None
                an = drain_dve(av0, av1)
                pend_tr = F(drain_pe, p, c, an)
            pend_tr()
            while fi < nf:
                fill[fi]()
                fi += 1

        # prologue: minimum needed for pair-0 slot 0 (+1 V-unit lookahead)
        for u in (F(k_unit, 0, 0), F(q_unit, 0, 0), F(v_unit, 0, 0),
                  F(v_unit, 0, 1)):
            u()
        # pair-0 fillers are deadline-exact: one after each of its 24 slots
        fill0 = [F(v_unit, 0, 2), F(v_unit, 0, 3), F(k_unit, 0, 2),
                 F(v_unit, 0, 4), F(v_unit, 0, 5), F(v_unit, 0, 6),
                 F(v_unit, 0, 7), F(q_unit, 0, 1), F(k_unit, 0, 1),
                 F(v_unit, 0, 8), F(v_unit, 0, 9), F(v_unit, 0, 10),
                 F(v_unit, 0, 11), F(k_unit, 0, 3),
                 F(v_unit, 0, 12), F(v_unit, 0, 13), F(v_unit, 0, 14),
                 F(v_unit, 0, 15),
                 F(k_unit, 1, 0), F(q_unit, 1, 0), F(k_unit, 1, 1),
                 F(k_unit, 1, 2), F(k_unit, 1, 3), F(q_unit, 1, 1)]
        filler_map = {
            0: fill0,
            1: pair_units(2),
            2: [F(v_unit, 1, i) for i in range(8)] + pair_units(3),
            3: [F(v_unit, 1, i) for i in range(8, 16)] + pair_units(4),
            4: pair_units(5),
            5: pair_units(6),
            6: pair_units(7),
            7: [],
        }
        for p in range(8):
            emit_pair(p, filler_map[p])


def _phase2_proj(nc, tc, xr, x1b, at, wo, bo):
    """x1 = x + attn @ Wo + bo, written in place into xr."""
    with tc.tile_pool(name="ps_y", bufs=4, space="PSUM") as pp_y:
        for j in range(ET):
            for c in range(2):
                ps = pp_y.tile([128, CH], F32)
                for i in range(ET):
                    nc.tensor.matmul(
                        ps[:], wo[:, i, j, :], at[:, i, c * CH:(c + 1) * CH],
                        start=(i == 0), stop=(i == ET - 1))
                nc.vector.scalar_tensor_tensor(
                    xr[:, j, c * CH:(c + 1) * CH], ps[:], bo[:, j, :],
                    xr[:, j, c * CH:(c + 1) * CH],
                    op0=mybir.AluOpType.add, op1=mybir.AluOpType.add)
                nc.vector.tensor_copy(
                    x1b[:, j, c * CH:(c + 1) * CH],
                    xr[:, j, c * CH:(c + 1) * CH])


def _phase3_ffn(nc, tc, xr, x1b, out_d, dram):
    w1_d, w2_d = dram["w1"], dram["w2"]
    with (
        tc.tile_pool(name="hT", bufs=2) as h_pool,
        tc.tile_pool(name="w1s", bufs=3) as w1_pool,
        tc.tile_pool(name="w2s", bufs=2) as w2_pool,
        tc.tile_pool(name="b12", bufs=1) as b12_pool,
        tc.tile_pool(name="osb", bufs=3) as o_pool,
        tc.tile_pool(name="ps_h", bufs=4, space="PSUM") as pp_h,
        tc.tile_pool(name="ps_f", bufs=4, space="PSUM") as pp_f,
    ):
        b1 = b12_pool.tile([128, HT, 1], F32, tag="b1")
        nc.sync.dma_start(b1[:], dram["b1"][:])
        b2 = b12_pool.tile([128, ET, 1], F32, tag="b2")
        nc.sync.dma_start(b2[:], dram["b2"][:])
        hT0 = h_pool.tile([128, HT, CH], BF16, tag="hT")
        hT1 = h_pool.tile([128, HT, CH], BF16, tag="hT")
        hts = [hT0, hT1]
        for t in range(HT):
            w1t = w1_pool.tile([128, ET, 128], BF16)
            nc.sync.dma_start(w1t[:], w1_d[t])
            for c in range(2):
                ps = pp_h.tile([128, CH], F32)
                for i in range(ET):
                    nc.tensor.matmul(
                        ps[:], w1t[:, i, :], x1b[:, i, c * CH:(c + 1) * CH],
                        start=(i == 0), stop=(i == ET - 1))
                nc.scalar.activation(
                    hts[c][:, t, :], ps[:], mybir.ActivationFunctionType.Relu,
                    bias=b1[:, t, :])
        for j in range(ET):
            w2t = w2_pool.tile([128, HT, 128], BF16)
            nc.sync.dma_start(w2t[:], w2_d[j])
            for c in range(2):
                ps = pp_f.tile([128, CH], F32)
                for t in range(HT):
                    nc.tensor.matmul(
                        ps[:], w2t[:, t, :], hts[c][:, t, :],
                        start=(t == 0), stop=(t == HT - 1))
                ot = o_pool.tile([128, CH], F32)
                nc.vector.scalar_tensor_tensor(
                    ot[:], ps[:], b2[:, j, :], xr[:, j, c * CH:(c + 1) * CH],
                    op0=mybir.AluOpType.add, op1=mybir.AluOpType.add)
                nc.sync.dma_start(out_d[j][:, c * CH:(c + 1) * CH], ot[:])


def build_nc(reps=1, phases=(1, 2, 3)):
    nc = bacc.Bacc("TRN2", target_bir_lowering=False, debug=False, num_devices=8)

    dram = {}
    dram["xT"] = nc.declare_dram_parameter("xT", [ET, 128, S], BF16, isOutput=False)
    dram["xr"] = nc.declare_dram_parameter("xr", [ET, 128, QC], F32, isOutput=False)
    dram["wq"] = nc.declare_dram_parameter("wq", [8, 128, ET, 128], BF16, isOutput=False)
    dram["wk"] = nc.declare_dram_parameter("wk", [8, 128, ET, 128], BF16, isOutput=False)
    dram["wv"] = nc.declare_dram_parameter("wv", [2, 128, ET, 512], BF16, isOutput=False)
    dram["wo"] = nc.declare_dram_parameter("wo", [128, ET, ET, 128], BF16, isOutput=False)
    dram["w1"] = nc.declare_dram_parameter("w1", [HT, 128, ET, 128], BF16, isOutput=False)
    dram["w2"] = nc.declare_dram_parameter("w2", [ET, 128, HT, 128], BF16, isOutput=False)
    dram["bo"] = nc.declare_dram_parameter("bo", [128, ET, 1], F32, isOutput=False)
    dram["b1"] = nc.declare_dram_parameter("b1", [128, HT, 1], F32, isOutput=False)
    dram["b2"] = nc.declare_dram_parameter("b2", [128, ET, 1], F32, isOutput=False)
    dram["masks"] = nc.declare_dram_parameter(
        "masks", [128, N_MASKS, CH], BF16, isOutput=False)
    out_d = nc.declare_dram_parameter("outT", [ET, 128, QC], F32, isOutput=True)

    with tile.TileContext(nc) as tc:
        for _rep in range(reps):
            with (
                tc.tile_pool(name="xt", bufs=1) as xt_pool,
                tc.tile_pool(name="x1b", bufs=1) as x1b_pool,
                tc.tile_pool(name="attnT", bufs=1) as at_pool,
            ):
                xt = xt_pool.tile([128, ET, S], BF16)
                x1b = x1b_pool.tile([128, ET, QC], BF16)
                at = at_pool.tile([128, ET, QC], BF16)
                with (
                    tc.tile_pool(name="masks", bufs=1) as mk_pool,
                    tc.tile_pool(name="wpre", bufs=1) as wpre_pool,
                ):
                    mk = mk_pool.tile([128, N_MASKS, CH], BF16)
                    # DMA order drives the critical path: pair-0 K/Q weights
                    # first, then full-width x tiles (one DMA per e-tile);
                    # masks ride the ACT hwdge ring in parallel.
                    nc.scalar.dma_start(mk[:], dram["masks"][:])
                    preload = {}
                    for kind in ("wk", "wq"):
                        wt = wpre_pool.tile([128, ET, 128], BF16, tag=f"{kind}0")
                        nc.sync.dma_start(wt[:], dram[kind][0])
                        preload[(kind, 0)] = wt
                    for et in range(ET):
                        nc.sync.dma_start(xt[:, et, :], dram["xT"][et][:, :])
                    for kind in ("wk", "wq"):
                        wt = wpre_pool.tile([128, ET, 128], BF16, tag=f"{kind}1")
                        nc.sync.dma_start(wt[:], dram[kind][1])
                        preload[(kind, 1)] = wt
                    if 1 in phases:
                        _phase1_attention(nc, tc, xt, at, mk, dram, preload)
                with (
                    tc.tile_pool(name="xr", bufs=1) as xr_pool,
                    tc.tile_pool(name="wo", bufs=1) as wo_pool,
                    tc.tile_pool(name="bo", bufs=1) as bo_pool,
                ):
                    xr = xr_pool.tile([128, ET, QC], F32)
                    wo = wo_pool.tile([128, ET, ET, 128], BF16)
                    bo = bo_pool.tile([128, ET, 1], F32)
                    for et in range(ET):
                        nc.sync.dma_start(xr[:, et, :], dram["xr"][et])
                    nc.sync.dma_start(wo[:], dram["wo"][:])
                    nc.sync.dma_start(bo[:], dram["bo"][:])
                    if 2 in phases:
                        _phase2_proj(nc, tc, xr, x1b, at, wo, bo)
                    if 3 in phases:
                        if 2 not in phases:
                            for et in range(ET):
                                nc.sync.dma_start(
                                    x1b[:, et, :], dram["xT"][et][:, 0:QC])
                        _phase3_ffn(nc, tc, xr, x1b, out_d, dram)

    nc.compile()
    return nc


def _qsel(half):
    if half == 0:
        return np.concatenate([np.arange(0, 512), np.arange(1536, 2048)])
    return np.arange(512, 1536)


def make_masks(half):
    """bf16 [128, 16, 1024] per-core causal keep-masks (dup for head pair)."""
    own = _qsel(half)
    other = _qsel(1 - half)
    tpos = np.concatenate([own, other])          # actual seq position per t col
    qpos = own
    m = np.zeros((N_MASKS, 128, CH), dtype=np.float32)
    for slots, q0 in ((CHUNK_A, 0), (CHUNK_B, 512)):
        for tt, mi in slots:
            if mi is None:
                continue
            q_act = qpos[q0:q0 + CH]
            t_act = tpos[tt * 128:(tt + 1) * 128]
            m[mi] = (t_act[:, None] <= q_act[None, :]).astype(np.float32)
    return np.ascontiguousarray(m.transpose(1, 0, 2)).astype(ml_dtypes.bfloat16)


def prep_shared(Wq, Wk, Wv, Wo, bo, W1, b1, W2, b2):
    f = np.float32
    wq = np.stack([Wq[2 * p:2 * p + 2].transpose(1, 0, 2).reshape(E, 128)
                   .reshape(ET, 128, 128).transpose(1, 0, 2) for p in range(8)])
    wk = np.stack([Wk[2 * p:2 * p + 2].transpose(1, 0, 2).reshape(E, 128)
                   .reshape(ET, 128, 128).transpose(1, 0, 2) for p in range(8)])
    wv = np.stack([Wv[8 * h:8 * h + 8].transpose(1, 0, 2).reshape(E, 512)
                   .reshape(ET, 128, 512).transpose(1, 0, 2) for h in range(2)])
    wo = Wo.reshape(ET, 128, ET, 128).transpose(1, 0, 2, 3)
    w1 = W1.reshape(ET, 128, HT, 128).transpose(2, 1, 0, 3)
    w2 = W2.reshape(HT, 128, ET, 128).transpose(2, 1, 0, 3)
    return {
        "wq": np.ascontiguousarray(wq).astype(ml_dtypes.bfloat16),
        "wk": np.ascontiguousarray(wk).astype(ml_dtypes.bfloat16),
        "wv": np.ascontiguousarray(wv).astype(ml_dtypes.bfloat16),
        "wo": np.ascontiguousarray(wo).astype(ml_dtypes.bfloat16),
        "w1": np.ascontiguousarray(w1).astype(ml_dtypes.bfloat16),
        "w2": np.ascontiguousarray(w2).astype(ml_dtypes.bfloat16),
        "bo": np.ascontiguousarray(bo.reshape(ET, 128, 1).transpose(1, 0, 2)).astype(f),
        "b1": np.ascontiguousarray(b1.reshape(HT, 128, 1).transpose(1, 0, 2)).astype(f),
        "b2": np.ascontiguousarray(b2.reshape(ET, 128, 1).transpose(1, 0, 2)).astype(f),
    }


def make_in_maps(x, Wq, Wk, Wv, Wo, bo, W1, b1, W2, b2):
    shared = prep_shared(Wq, Wk, Wv, Wo, bo, W1, b1, W2, b2)
    masks = [make_masks(half) for half in range(2)]
    in_maps = []
    for core in range(8):
        b, half = core // 2, core % 2
        own = _qsel(half)
        torder = np.concatenate([own, _qsel(1 - half)])
        xTc = np.ascontiguousarray(
            np.asarray(x[b]).T[:, torder]).reshape(ET, 128, S)
        in_maps.append({"xT": xTc.astype(ml_dtypes.bfloat16),
                        "xr": xTc[:, :, 0:QC].astype(np.float32),
                        "masks": masks[half], **shared})
    return in_maps


def kernel(**inputs):
    global LAST_RESULTS
    if "nc" not in _CACHE:
        _CACHE["nc"] = build_nc()
    nc = _CACHE["nc"]
    in_maps = make_in_maps(
        inputs["x"], inputs["Wq"], inputs["Wk"], inputs["Wv"], inputs["Wo"],
        inputs["bo"], inputs["W1"], inputs["b1"], inputs["W2"], inputs["b2"])
    res = run_bass_kernel_spmd(nc, in_maps, list(range(8)))
    LAST_RESULTS = res
    out = np.empty((B, S, E), dtype=np.float32)
    for core in range(8):
        b, half = core // 2, core % 2
        outT = res.results[core]["outT"].reshape(E, QC)
        out[b, _qsel(half), :] = outT.T
    return out



# revision 27
# speedup vs baseline: 1.0232x; 1.0232x over previous
"""Trainium2 Bass kernel for nn_DecoderBlock (B=4, S=2048, E=1024, H=16, D=64).

Sharding: 8 cores = 4 batches x 2 sequence-halves. Each core owns 1024 query
positions of one batch (a balanced causal split: core-even takes q [0:512)+
[1536:2048), core-odd takes q [512:1536)) and recomputes full-S K/V for its
batch locally (no collectives). Proj + FFN are token-parallel on the owned
1024 positions. Everything on-chip is in transposed layout (feature dim on
partitions); the host pre-transposes x and re-transposes the output.

The per-core program is identical (SPMD); per-core differences (which q
columns, causal masks) are encoded in the host-prepared inputs: xT columns
are reordered to [own-q | other-q], and causal masks are shipped per-core.
"""

import numpy as np
import ml_dtypes

import concourse.bass as bass
import concourse.tile as tile
from concourse import bacc, mybir
from concourse.bass_utils import run_bass_kernel_spmd

B, S, E, H, D = 4, 2048, 1024, 16, 64
QC = 1024          # queries owned per core
CH = 512           # q-chunk (matmul moving dim)
ET = E // 128      # 8 e-tiles
HT = (4 * E) // 128  # 32 ffn hidden tiles
SCALE = float(E) ** -0.5

F32R = mybir.dt.float32r
F32 = mybir.dt.float32
BF16 = mybir.dt.bfloat16

# Attention slot tables: (t_tile, mask_idx or None); uniform across cores.
# xT t-order is [own qA | own qB | other qA | other qB] (512 cols each).
# Each slot computes BOTH heads of a pair concurrently via PE row-groups
# (rows 0-63 / 64-127) into one [128,1024] psum tile -> one exp op.
CHUNK_A = [(0, 0), (1, 1), (2, 2), (3, 3), (8, 4), (9, 5), (10, 6), (11, 7)]
CHUNK_B = [(0, None), (1, None), (2, None), (3, None),
           (4, 8), (5, 9), (6, 10), (7, 11),
           (8, None), (9, None), (10, None), (11, None),
           (12, 12), (13, 13), (14, 14), (15, 15)]
N_MASKS = 16

_CACHE = {}
LAST_RESULTS = None


# V-unit emission order per half: chunk-A slot order first, then chunk-B extras
V_ORD = [0, 1, 2, 3, 8, 9, 10, 11, 4, 5, 6, 7, 12, 13, 14, 15]


def _phase1_attention(nc, tc, xt, at, mk, dram, preload=None):
    """Attention with interleaved K/Q/V filler units inside the slot stream.

    Per slot: two per-head scores matmuls [128t,512q] -> exp -> mask; AV runs
    with exp-scores as the STATIONARY operand (out [128q, 65], moving dim 65 =
    64 v-features + ones column for the softmax denominator), accumulated over
    slots per q-subtile. Chunk drain: reciprocal + per-subtile scale, then PE
    transpose back to [feat, q] into at. K/Q/V units for later pairs are
    emitted between slots so the PE never starves while ACT runs exp.
    """
    from functools import partial as F
    from concourse.masks import make_identity

    with (
        tc.tile_pool(name="wkq", bufs=3) as wkq_pool,
        tc.tile_pool(name="wv", bufs=2) as wv_pool,
        tc.tile_pool(name="kt", bufs=3) as kt_pool,
        tc.tile_pool(name="qt", bufs=3) as qt_pool,
        tc.tile_pool(name="vt", bufs=2) as vt_pool,
        tc.tile_pool(name="es", bufs=3) as es_pool,
        tc.tile_pool(name="an", bufs=2) as an_pool,
        tc.tile_pool(name="norm", bufs=2) as nm_pool,
        tc.tile_pool(name="ident", bufs=1) as id_pool,
        tc.tile_pool(name="ps_kqv", bufs=2, space="PSUM") as pp_kqv,
        tc.tile_pool(name="ps_s", bufs=2, space="PSUM") as pp_s,
        tc.tile_pool(name="ps_av", bufs=2, space="PSUM") as pp_av,
    ):
        ident = id_pool.tile([128, 128], BF16)
        make_identity(nc, ident[:])

        kts, qts, wks, wqs, vts, wvs = {}, {}, {}, {}, {}, {}

        def k_unit(p, tcnk):
            if tcnk == 0:
                if preload and ("wk", p) in preload:
                    wks[p] = preload[("wk", p)]
                else:
                    wks[p] = wkq_pool.tile([128, ET, 128], BF16, tag="w", name=f"wk{p}")
                    nc.sync.dma_start(wks[p][:], dram["wk"][p])
                kts[p] = kt_pool.tile([128, S], BF16, tag="kt", name=f"kt{p}")
            ps = pp_kqv.tile([128, CH], F32)
            for et in range(ET):
                nc.tensor.matmul(
                    ps[:], wks[p][:, et, :], xt[:, et, tcnk * CH:(tcnk + 1) * CH],
                    start=(et == 0), stop=(et == ET - 1))
            nc.vector.tensor_copy(kts[p][:, tcnk * CH:(tcnk + 1) * CH], ps[:])

        def q_unit(p, c):
            if c == 0:
                if preload and ("wq", p) in preload:
                    wqs[p] = preload[("wq", p)]
                else:
                    wqs[p] = wkq_pool.tile([128, ET, 128], BF16, tag="w", name=f"wq{p}")
                    nc.sync.dma_start(wqs[p][:], dram["wq"][p])
                qts[p] = qt_pool.tile([128, QC], BF16, tag="qt", name=f"qt{p}")
            ps = pp_kqv.tile([128, CH], F32)
            for et in range(ET):
                nc.tensor.matmul(
                    ps[:], wqs[p][:, et, :], xt[:, et, c * CH:(c + 1) * CH],
                    start=(et == 0), stop=(et == ET - 1))
            nc.vector.tensor_copy(qts[p][:, c * CH:(c + 1) * CH], ps[:])

        def v_unit(half, i):
            tt = V_ORD[i]
            if i == 0:
                wvs[half] = wv_pool.tile([128, ET, 512], BF16, tag="wv", name=f"wv{half}")
                nc.sync.dma_start(wvs[half][:], dram["wv"][half])
                vts[half] = vt_pool.tile([128, 16, 8, 65], BF16, tag="vt", name=f"vt{half}")
                nc.vector.memset(vts[half][:, :, :, 64:65], 1.0)
            ps = pp_kqv.tile([128, CH], F32)
            for et in range(ET):
                nc.tensor.matmul(
                    ps[:], xt[:, et, tt * 128:(tt + 1) * 128], wvs[half][:, et, :],
                    start=(et == 0), stop=(et == ET - 1))
            nc.vector.tensor_copy(
                vts[half][:, tt, :, 0:64],
                ps[:].rearrange("p (h d) -> p h d", h=8))

        def slot_front(p, c, tt, mi):
            """Scores + exp + masks for one slot; returns the es tile."""
            kt, qt = kts[p], qts[p]
            ps = pp_s.tile([128, 2 * CH], F32, tag="ps")
            nc.tensor.matmul(
                ps[:, 0:CH], kt[0:64, tt * 128:(tt + 1) * 128],
                qt[0:64, c * CH:(c + 1) * CH], start=True, stop=True)
            nc.tensor.matmul(
                ps[:, CH:2 * CH], kt[64:128, tt * 128:(tt + 1) * 128],
                qt[64:128, c * CH:(c + 1) * CH], start=True, stop=True)
            es = es_pool.tile([128, 2 * CH], BF16)
            nc.scalar.activation(
                es[:], ps[:], mybir.ActivationFunctionType.Exp, scale=SCALE)
            if mi is not None:
                nc.vector.tensor_mul(es[:, 0:CH], es[:, 0:CH], mk[:, mi, :])
                nc.vector.tensor_mul(
                    es[:, CH:2 * CH], es[:, CH:2 * CH], mk[:, mi, :])
            return es

        def slot_avs(p, si, n, es, tt, av0, av1):
            # One accumulation group per av bank: start zeroes the whole 2KB
            # zero region, so only (si=0, sub=0) starts and (last si, sub=3)
            # stops; the 4 subtile chains share the group.
            vt = vts[p // 4]
            hh0, hh1 = 2 * (p % 4), 2 * (p % 4) + 1
            for sub in range(4):
                st = si == 0 and sub == 0
                sp = si == n - 1 and sub == 3
                nc.tensor.matmul(
                    av0[:, sub, 0:65], es[:, sub * 128:(sub + 1) * 128],
                    vt[:, tt, hh0, :], start=st, stop=sp, skip_group_check=True)
                nc.tensor.matmul(
                    av1[:, sub, 0:65], es[:, CH + sub * 128:CH + (sub + 1) * 128],
                    vt[:, tt, hh1, :], start=st, stop=sp, skip_group_check=True)

        def drain_dve(av0, av1):
            """Normalize: reciprocal of denominators + scale to bf16 an."""
            rc = nm_pool.tile([128, 2, 4], F32, tag="rc")
            nc.vector.reciprocal(rc[:, 0, :], av0[:, :, 64])
            nc.vector.reciprocal(rc[:, 1, :], av1[:, :, 64])
            an = an_pool.tile([128, 4, 2, 64], BF16)
            for sub in range(4):
                nc.vector.tensor_scalar_mul(
                    an[:, sub, 0, :], av0[:, sub, 0:64], rc[:, 0, sub:sub + 1])
                nc.vector.tensor_scalar_mul(
                    an[:, sub, 1, :], av1[:, sub, 0:64], rc[:, 1, sub:sub + 1])
            return an

        def drain_pe(p, c, an):
            """Transpose an back to [feat, q] and store into at."""
            for sub in range(4):
                tr = pp_kqv.tile([128, 128], BF16, tag="ps", name="tr")
                nc.tensor.transpose(
                    tr[:], an[:, sub].rearrange("p h d -> p (h d)"), ident[:])
                nc.vector.tensor_copy(
                    at[:, p, c * CH + sub * 128:c * CH + (sub + 1) * 128], tr[:])

        def pair_units(p):
            return [F(k_unit, p, 0), F(q_unit, p, 0), F(k_unit, p, 1),
                    F(k_unit, p, 2), F(k_unit, p, 3), F(q_unit, p, 1)]

        def emit_pair(p, fill):
            fi, nf = 0, len(fill)
            pend_avs, pend_tr = None, None
            for c, slots in ((0, CHUNK_A), (1, CHUNK_B)):
                av0 = pp_av.tile([128, 4, 128], F32, tag="av")
                av1 = pp_av.tile([128, 4, 128], F32, tag="av")
                n = len(slots)
                gbase = 0 if c == 0 else len(CHUNK_A)
                for si, (tt, mi) in enumerate(slots):
                    es = slot_front(p, c, tt, mi)
                    if pend_avs is not None:
                        pend_avs()
                        pend_avs = None
                    if pend_tr is not None:
                        pend_tr()
                        pend_tr = None
                    pend_avs = F(slot_avs, p, si, n, es, tt, av0, av1)
                    due = min(nf, ((gbase + si + 1) * nf) // 24)
                    while fi < due:
                        fill[fi]()
                        fi += 1
                pend_avs()
                pend_avs = None
                an = drain_dve(av0, av1)
                pend_tr = F(drain_pe, p, c, an)
            pend_tr()
            while fi < nf:
                fill[fi]()
                fi += 1

        # prologue: minimum needed for pair-0 slot 0 (+1 V-unit lookahead)
        for u in (F(k_unit, 0, 0), F(q_unit, 0, 0), F(v_unit, 0, 0),
                  F(v_unit, 0, 1)):
            u()
        # pair-0 fillers are deadline-exact: one after each of its 24 slots
        fill0 = [F(v_unit, 0, 2), F(v_unit, 0, 3), F(k_unit, 0, 2),
                 F(v_unit, 0, 4), F(v_unit, 0, 5), F(v_unit, 0, 6),
                 F(v_unit, 0, 7), F(q_unit, 0, 1), F(k_unit, 0, 1),
                 F(v_unit, 0, 8), F(v_unit, 0, 9), F(v_unit, 0, 10),
                 F(v_unit, 0, 11), F(k_unit, 0, 3),
                 F(v_unit, 0, 12), F(v_unit, 0, 13), F(v_unit, 0, 14),
                 F(v_unit, 0, 15),
                 F(k_unit, 1, 0), F(q_unit, 1, 0), F(k_unit, 1, 1),
                 F(k_unit, 1, 2), F(k_unit, 1, 3), F(q_unit, 1, 1)]
        filler_map = {
            0: fill0,
            1: pair_units(2),
            2: [F(v_unit, 1, i) for i in range(8)] + pair_units(3),
            3: [F(v_unit, 1, i) for i in range(8, 16)] + pair_units(4),
            4: pair_units(5),
            5: pair_units(6),
            6: pair_units(7),
            7: [],
        }
        for p in range(8):
            emit_pair(p, filler_map[p])


def _phase2_proj(nc, tc, xr, x1b, at, wo, bo):
    """x1 = x + attn @ Wo + bo, written in place into xr."""
    with tc.tile_pool(name="ps_y", bufs=4, space="PSUM") as pp_y:
        for j in range(ET):
            for c in range(2):
                ps = pp_y.tile([128, CH], F32)
                for i in range(ET):
                    nc.tensor.matmul(
                        ps[:], wo[:, i, j, :], at[:, i, c * CH:(c + 1) * CH],
                        start=(i == 0), stop=(i == ET - 1))
                nc.vector.scalar_tensor_tensor(
                    xr[:, j, c * CH:(c + 1) * CH], ps[:], bo[:, j, :],
                    xr[:, j, c * CH:(c + 1) * CH],
                    op0=mybir.AluOpType.add, op1=mybir.AluOpType.add)
                nc.vector.tensor_copy(
                    x1b[:, j, c * CH:(c + 1) * CH],
                    xr[:, j, c * CH:(c + 1) * CH])


def _phase3_ffn(nc, tc, xr, x1b, out_d, dram):
    w1_d, w2_d = dram["w1"], dram["w2"]
    with (
        tc.tile_pool(name="hT", bufs=2) as h_pool,
        tc.tile_pool(name="w1s", bufs=3) as w1_pool,
        tc.tile_pool(name="w2s", bufs=2) as w2_pool,
        tc.tile_pool(name="b12", bufs=1) as b12_pool,
        tc.tile_pool(name="osb", bufs=3) as o_pool,
        tc.tile_pool(name="ps_h", bufs=4, space="PSUM") as pp_h,
        tc.tile_pool(name="ps_f", bufs=4, space="PSUM") as pp_f,
    ):
        b1 = b12_pool.tile([128, HT, 1], F32, tag="b1")
        nc.sync.dma_start(b1[:], dram["b1"][:])
        b2 = b12_pool.tile([128, ET, 1], F32, tag="b2")
        nc.sync.dma_start(b2[:], dram["b2"][:])
        hT0 = h_pool.tile([128, HT, CH], BF16, tag="hT")
        hT1 = h_pool.tile([128, HT, CH], BF16, tag="hT")
        hts = [hT0, hT1]
        for t in range(HT):
            w1t = w1_pool.tile([128, ET, 128], BF16)
            nc.sync.dma_start(w1t[:], w1_d[t])
            for c in range(2):
                ps = pp_h.tile([128, CH], F32)
                for i in range(ET):
                    nc.tensor.matmul(
                        ps[:], w1t[:, i, :], x1b[:, i, c * CH:(c + 1) * CH],
                        start=(i == 0), stop=(i == ET - 1))
                nc.scalar.activation(
                    hts[c][:, t, :], ps[:], mybir.ActivationFunctionType.Relu,
                    bias=b1[:, t, :])
        for j in range(ET):
            w2t = w2_pool.tile([128, HT, 128], BF16)
            nc.sync.dma_start(w2t[:], w2_d[j])
            for c in range(2):
                ps = pp_f.tile([128, CH], F32)
                for t in range(HT):
                    nc.tensor.matmul(
                        ps[:], w2t[:, t, :], hts[c][:, t, :],
                        start=(t == 0), stop=(t == HT - 1))
                ot = o_pool.tile([128, CH], F32)
                nc.vector.scalar_tensor_tensor(
                    ot[:], ps[:], b2[:, j, :], xr[:, j, c * CH:(c + 1) * CH],
                    op0=mybir.AluOpType.add, op1=mybir.AluOpType.add)
                nc.sync.dma_start(out_d[j][:, c * CH:(c + 1) * CH], ot[:])


def build_nc(reps=1, phases=(1, 2, 3)):
    nc = bacc.Bacc("TRN2", target_bir_lowering=False, debug=False, num_devices=8)

    dram = {}
    dram["xT"] = nc.declare_dram_parameter("xT", [ET, 128, S], BF16, isOutput=False)
    dram["xr"] = nc.declare_dram_parameter("xr", [ET, 128, QC], F32, isOutput=False)
    dram["wq"] = nc.declare_dram_parameter("wq", [8, 128, ET, 128], BF16, isOutput=False)
    dram["wk"] = nc.declare_dram_parameter("wk", [8, 128, ET, 128], BF16, isOutput=False)
    dram["wv"] = nc.declare_dram_parameter("wv", [2, 128, ET, 512], BF16, isOutput=False)
    dram["wo"] = nc.declare_dram_parameter("wo", [128, ET, ET, 128], BF16, isOutput=False)
    dram["w1"] = nc.declare_dram_parameter("w1", [HT, 128, ET, 128], BF16, isOutput=False)
    dram["w2"] = nc.declare_dram_parameter("w2", [ET, 128, HT, 128], BF16, isOutput=False)
    dram["bo"] = nc.declare_dram_parameter("bo", [128, ET, 1], F32, isOutput=False)
    dram["b1"] = nc.declare_dram_parameter("b1", [128, HT, 1], F32, isOutput=False)
    dram["b2"] = nc.declare_dram_parameter("b2", [128, ET, 1], F32, isOutput=False)
    dram["masks"] = nc.declare_dram_parameter(
        "masks", [128, N_MASKS, CH], BF16, isOutput=False)
    out_d = nc.declare_dram_parameter("outT", [ET, 128, QC], F32, isOutput=True)

    with tile.TileContext(nc) as tc:
        for _rep in range(reps):
            with (
                tc.tile_pool(name="xt", bufs=1) as xt_pool,
                tc.tile_pool(name="x1b", bufs=1) as x1b_pool,
                tc.tile_pool(name="attnT", bufs=1) as at_pool,
            ):
                xt = xt_pool.tile([128, ET, S], BF16)
                x1b = x1b_pool.tile([128, ET, QC], BF16)
                at = at_pool.tile([128, ET, QC], BF16)
                with (
                    tc.tile_pool(name="masks", bufs=1) as mk_pool,
                    tc.tile_pool(name="wpre", bufs=1) as wpre_pool,
                ):
                    mk = mk_pool.tile([128, N_MASKS, CH], BF16)
                    # DMA order drives the critical path: pair-0 K/Q weights
                    # first, then full-width x tiles (one DMA per e-tile);
                    # masks ride the ACT hwdge ring in parallel.
                    nc.scalar.dma_start(mk[:], dram["masks"][:])
                    preload = {}
                    for kind in ("wk", "wq"):
                        wt = wpre_pool.tile([128, ET, 128], BF16, tag=f"{kind}0")
                        nc.sync.dma_start(wt[:], dram[kind][0])
                        preload[(kind, 0)] = wt
                    for et in range(ET):
                        nc.sync.dma_start(xt[:, et, :], dram["xT"][et][:, :])
                    for kind in ("wk", "wq"):
                        wt = wpre_pool.tile([128, ET, 128], BF16, tag=f"{kind}1")
                        nc.sync.dma_start(wt[:], dram[kind][1])
                        preload[(kind, 1)] = wt
                    if 1 in phases:
                        _phase1_attention(nc, tc, xt, at, mk, dram, preload)
                with (
                    tc.tile_pool(name="xr", bufs=1) as xr_pool,
                    tc.tile_pool(name="wo", bufs=1) as wo_pool,
                    tc.tile_pool(name="bo", bufs=1) as bo_pool,
                ):
                    xr = xr_pool.tile([128, ET, QC], F32)
                    wo = wo_pool.tile([128, ET, ET, 128], BF16)
                    bo = bo_pool.tile([128, ET, 1], F32)
                    nc.sync.dma_start(wo[:], dram["wo"][:])
                    nc.sync.dma_start(bo[:], dram["bo"][:])
                    for et in range(ET):
                        nc.sync.dma_start(xr[:, et, :], dram["xr"][et])
                    if 2 in phases:
                        _phase2_proj(nc, tc, xr, x1b, at, wo, bo)
                    if 3 in phases:
                        if 2 not in phases:
                            for et in range(ET):
                                nc.sync.dma_start(
                                    x1b[:, et, :], dram["xT"][et][:, 0:QC])
                        _phase3_ffn(nc, tc, xr, x1b, out_d, dram)

    nc.compile()
    return nc


def _qsel(half):
    if half == 0:
        return np.concatenate([np.arange(0, 512), np.arange(1536, 2048)])
    return np.arange(512, 1536)


def make_masks(half):
    """bf16 [128, 16, 1024] per-core causal keep-masks (dup for head pair)."""
    own = _qsel(half)
    other = _qsel(1 - half)
    tpos = np.concatenate([own, other])          # actual seq position per t col
    qpos = own
    m = np.zeros((N_MASKS, 128, CH), dtype=np.float32)
    for slots, q0 in ((CHUNK_A, 0), (CHUNK_B, 512)):
        for tt, mi in slots:
            if mi is None:
                continue
            q_act = qpos[q0:q0 + CH]
            t_act = tpos[tt * 128:(tt + 1) * 128]
            m[mi] = (t_act[:, None] <= q_act[None, :]).astype(np.float32)
    return np.ascontiguousarray(m.transpose(1, 0, 2)).astype(ml_dtypes.bfloat16)


def prep_shared(Wq, Wk, Wv, Wo, bo, W1, b1, W2, b2):
    f = np.float32
    wq = np.stack([Wq[2 * p:2 * p + 2].transpose(1, 0, 2).reshape(E, 128)
                   .reshape(ET, 128, 128).transpose(1, 0, 2) for p in range(8)])
    wk = np.stack([Wk[2 * p:2 * p + 2].transpose(1, 0, 2).reshape(E, 128)
                   .reshape(ET, 128, 128).transpose(1, 0, 2) for p in range(8)])
    wv = np.stack([Wv[8 * h:8 * h + 8].transpose(1, 0, 2).reshape(E, 512)
                   .reshape(ET, 128, 512).transpose(1, 0, 2) for h in range(2)])
    wo = Wo.reshape(ET, 128, ET, 128).transpose(1, 0, 2, 3)
    w1 = W1.reshape(ET, 128, HT, 128).transpose(2, 1, 0, 3)
    w2 = W2.reshape(HT, 128, ET, 128).transpose(2, 1, 0, 3)
    return {
        "wq": np.ascontiguousarray(wq).astype(ml_dtypes.bfloat16),
        "wk": np.ascontiguousarray(wk).astype(ml_dtypes.bfloat16),
        "wv": np.ascontiguousarray(wv).astype(ml_dtypes.bfloat16),
        "wo": np.ascontiguousarray(wo).astype(ml_dtypes.bfloat16),
        "w1": np.ascontiguousarray(w1).astype(ml_dtypes.bfloat16),
        "w2": np.ascontiguousarray(w2).astype(ml_dtypes.bfloat16),
        "bo": np.ascontiguousarray(bo.reshape(ET, 128, 1).transpose(1, 0, 2)).astype(f),
        "b1": np.ascontiguousarray(b1.reshape(HT, 128, 1).transpose(1, 0, 2)).astype(f),
        "b2": np.ascontiguousarray(b2.reshape(ET, 128, 1).transpose(1, 0, 2)).astype(f),
    }


def make_in_maps(x, Wq, Wk, Wv, Wo, bo, W1, b1, W2, b2):
    shared = prep_shared(Wq, Wk, Wv, Wo, bo, W1, b1, W2, b2)
    masks = [make_masks(half) for half in range(2)]
    in_maps = []
    for core in range(8):
        b, half = core // 2, core % 2
        own = _qsel(half)
        torder = np.concatenate([own, _qsel(1 - half)])
        xTc = np.ascontiguousarray(
            np.asarray(x[b]).T[:, torder]).reshape(ET, 128, S)
        in_maps.append({"xT": xTc.astype(ml_dtypes.bfloat16),
                        "xr": xTc[:, :, 0:QC].astype(np.float32),
                        "masks": masks[half], **shared})
    return in_maps


def kernel(**inputs):
    global LAST_RESULTS
    if "nc" not in _CACHE:
        _CACHE["nc"] = build_nc()
    nc = _CACHE["nc"]
    in_maps = make_in_maps(
        inputs["x"], inputs["Wq"], inputs["Wk"], inputs["Wv"], inputs["Wo"],
        inputs["bo"], inputs["W1"], inputs["b1"], inputs["W2"], inputs["b2"])
    res = run_bass_kernel_spmd(nc, in_maps, list(range(8)))
    LAST_RESULTS = res
    out = np.empty((B, S, E), dtype=np.float32)
    for core in range(8):
        b, half = core // 2, core % 2
        outT = res.results[core]["outT"].reshape(E, QC)
        out[b, _qsel(half), :] = outT.T
    return out

